# revision 3
# baseline (speedup 1.0000x reference)
"""Trainium2 Bass kernel for nn_DepthConsistencyLoss.

Problem: for each batch b (8 total), project 500k 3D points to a 256x256
pixel grid (pinhole projection with clipping), scatter-add conditional
density values into the grid, then compute a masked MSE loss against a
depth map, globally averaged over masked pixels of all batches.

Strategy: data-parallel over batch across the 8 NeuronCores (one batch per
core).  The scatter-add is done with a factorized one-hot matmul: for each
column of 128 points, build a val-weighted one-hot over p = v>>1 (128 wide,
the PE stationary operand) and a one-hot over j = (v&1)*256 + u (512 wide,
the PE moving operand); their product accumulated in PSUM is exactly the
[128, 512] = [256, 256] scatter grid.  The masked-MSE epilogue reduces the
grid to two scalars per core (sum of squared masked diffs, mask count); the
final cross-core reduction is two 8-element sums on the host.
"""

import sys

sys.path.insert(0, "/opt/trn_rl_repo")

import numpy as np

B = 8
N = 500000
H = W = 256
P = 128
COLS = 3908                      # ceil(N/128) -> padded point columns
NP = P * COLS                    # padded points per core
FT = 512                         # columns per processing tile
TILE_SIZES = [FT] * (COLS // FT) + ([COLS % FT] if COLS % FT else [])

_compiled = {}
last_results = None              # BassKernelResults of the most recent run


def _build_module(cols=COLS, tile_sizes=None):
    import concourse.bacc as bacc
    import concourse.tile as tile
    from concourse import mybir

    Alu = mybir.AluOpType
    f32 = mybir.dt.float32
    f16 = mybir.dt.float16
    i16 = mybir.dt.int16
    i32 = mybir.dt.int32

    import time as _time

    _t0 = _time.time()
    if tile_sizes is None:
        tile_sizes = [FT] * (cols // FT) + ([cols % FT] if cols % FT else [])
    nc = bacc.Bacc("TRN2", target_bir_lowering=False, debug=False)
    pts_ap = nc.dram_tensor("pts", [P, 4 * cols], f32, kind="ExternalInput").ap()
    dep_ap = nc.dram_tensor("dep", [P, 512], f32, kind="ExternalInput").ap()
    out_ap = nc.dram_tensor("out", [P, 2], f32, kind="ExternalOutput").ap()

    with tile.TileContext(nc) as tc:
        with (
            tc.tile_pool(name="const", bufs=1) as const,
            tc.tile_pool(name="io", bufs=3) as io,
            tc.tile_pool(name="work", bufs=2) as work,
            tc.tile_pool(name="ohp", bufs=8) as ohp,
            tc.tile_pool(name="ohj", bufs=8) as ohj,
            tc.tile_pool(name="psum", bufs=1, space="PSUM") as psum,
        ):
            iota_i = const.tile([P, 512], i16)
            nc.gpsimd.iota(iota_i[:], [[1, 512]], channel_multiplier=0)
            iotah = const.tile([P, 512], f16)
            nc.vector.tensor_copy(iotah[:], iota_i[:])
            dep_t = const.tile([P, 512], f32)
            nc.sync.dma_start(dep_t[:], dep_ap[:])

            grid = psum.tile([P, 512], f32, space="PSUM")

            chunk = 0
            nchunks = cols
            c0 = 0
            for ft in tile_sizes:
                x = io.tile([P, ft], f32, tag="x")
                nc.sync.dma_start(x[:], pts_ap[:, 0 * cols + c0 : 0 * cols + c0 + ft])
                y = io.tile([P, ft], f32, tag="y")
                nc.sync.dma_start(y[:], pts_ap[:, 1 * cols + c0 : 1 * cols + c0 + ft])
                z = io.tile([P, ft], f32, tag="z")
                nc.sync.dma_start(z[:], pts_ap[:, 2 * cols + c0 : 2 * cols + c0 + ft])
                d = io.tile([P, ft], f32, tag="d")
                nc.sync.dma_start(d[:], pts_ap[:, 3 * cols + c0 : 3 * cols + c0 + ft])

                rzs = work.tile([P, ft], f32, tag="rzs")
                rz = work.tile([P, ft], f32, tag="rz")
                nc.vector.reciprocal_approx_accurate(out=rz[:], in_=z[:], scratch=rzs[:])

                # u = floor(clip(x*256/z + 128, 0, 255)); same for v
                def proj_floor(w_, tagp):
                    t = work.tile([P, ft], f32, tag=tagp + "f")
                    nc.vector.scalar_tensor_tensor(t[:], w_[:], 256.0, rz[:], Alu.mult, Alu.mult)
                    tc_ = work.tile([P, ft], f32, tag=tagp + "c")
                    nc.vector.tensor_scalar(tc_[:], t[:], 128.0, 0.0, Alu.add, Alu.max)
                    tm = work.tile([P, ft], f32, tag=tagp + "m")
                    nc.vector.tensor_scalar(tm[:], tc_[:], 255.0, None, Alu.min)
                    tr = work.tile([P, ft], i32, tag=tagp + "r")
                    nc.vector.tensor_copy(tr[:], tm[:])          # RNE to int
                    trf = work.tile([P, ft], f32, tag=tagp + "rf")
                    nc.vector.tensor_copy(trf[:], tr[:])
                    tg = work.tile([P, ft], f32, tag=tagp + "g")
                    nc.vector.tensor_tensor(tg[:], trf[:], tm[:], Alu.is_gt)
                    tfl = work.tile([P, ft], f32, tag=tagp + "fl")
                    nc.vector.tensor_tensor(tfl[:], trf[:], tg[:], Alu.subtract)
                    return tfl                                   # exact floor, f32

                ufl = proj_floor(x, "u")
                vfl = proj_floor(y, "v")

                vi = work.tile([P, ft], i32, tag="vi")
                nc.vector.tensor_copy(vi[:], vfl[:])             # exact (integral)
                pi = work.tile([P, ft], i32, tag="pi")
                nc.vector.tensor_scalar(pi[:], vi[:], 1, None, Alu.logical_shift_right)
                oi = work.tile([P, ft], i32, tag="oi")
                nc.vector.tensor_scalar(oi[:], vi[:], 1, None, Alu.bitwise_and)
                pf = work.tile([P, ft], f32, tag="pf")
                nc.vector.tensor_copy(pf[:], pi[:])
                of = work.tile([P, ft], f32, tag="of")
                nc.vector.tensor_copy(of[:], oi[:])
                jf = work.tile([P, ft], f32, tag="jf")
                nc.vector.scalar_tensor_tensor(jf[:], of[:], 256.0, ufl[:], Alu.mult, Alu.add)

                zd = work.tile([P, ft], f32, tag="zd")
                nc.vector.tensor_tensor(zd[:], z[:], d[:], Alu.mult)
                val = work.tile([P, ft], f32, tag="val")
                nc.vector.scalar_tensor_tensor(val[:], d[:], 0.5, zd[:], Alu.is_gt, Alu.mult)

                for c in range(ft):
                    poh = ohp.tile([P, P], f16, tag="poh")
                    nc.vector.tensor_scalar(
                        poh[:], iotah[:, :P], pf[:, c : c + 1], val[:, c : c + 1],
                        Alu.is_equal, Alu.mult,
                    )
                    joh = ohj.tile([P, 512], f16, tag="joh")
                    nc.vector.tensor_scalar(
                        joh[:], iotah[:], jf[:, c : c + 1], None, Alu.is_equal,
                    )
                    nc.tensor.matmul(
                        grid[:], lhsT=poh[:], rhs=joh[:],
                        start=(chunk == 0), stop=(chunk == nchunks - 1),
                    )
                    chunk += 1
                c0 += ft

            # epilogue: masked MSE partials (per-partition; final sum on host)
            mask = work.tile([P, 512], f32, tag="mask")
            nc.vector.tensor_scalar(mask[:], grid[:], 0.0, None, Alu.is_gt)
            diff = work.tile([P, 512], f32, tag="diff")
            nc.vector.tensor_tensor(diff[:], grid[:], dep_t[:], Alu.subtract)
            diffm = work.tile([P, 512], f32, tag="diffm")
            nc.vector.tensor_tensor(diffm[:], diff[:], mask[:], Alu.mult)
            d2 = work.tile([P, 512], f32, tag="d2")
            nc.vector.tensor_tensor(d2[:], diffm[:], diffm[:], Alu.mult)

            res_sb = const.tile([P, 2], f32)

            def tree_reduce(src, col):
                w = 256
                cur = src
                while w >= 1:
                    nxt = work.tile([P, w], f32, tag="red")
                    nc.vector.tensor_tensor(
                        nxt[:], cur[:, :w], cur[:, w : 2 * w], Alu.add
                    )
                    cur = nxt
                    w //= 2
                nc.vector.tensor_copy(res_sb[:, col : col + 1], cur[:])

            tree_reduce(d2, 0)
            tree_reduce(mask, 1)
            nc.sync.dma_start(out_ap[:], res_sb[:])

    print(f"[kernel] tile trace+schedule: {_time.time() - _t0:.1f}s", flush=True)
    _t = _time.time()
    nc.compile()
    print(f"[kernel] bacc compile: {_time.time() - _t:.1f}s", flush=True)
    return nc


def _get_module():
    if "nc" not in _compiled:
        _compiled["nc"] = _build_module()
    return _compiled["nc"]


def _prepare_in_maps(points, densities, depth):
    points = np.asarray(points)
    densities = np.asarray(densities)
    depth = np.asarray(depth)
    in_maps = []
    for b in range(B):
        pts = np.empty((P, 4, COLS), np.float32)
        # layout: partition p holds [x | y | z | d], each a COLS-long row
        comp = np.ones((4, NP), np.float32)
        comp[0, :N] = points[b, :, 0]
        comp[1, :N] = points[b, :, 1]
        comp[2, :N] = points[b, :, 2]
        comp[2, N:] = 1.0
        comp[3, :N] = densities[b, :, 0]
        comp[3, N:] = 0.0
        comp[0, N:] = 0.0
        comp[1, N:] = 0.0
        pts[:, 0, :] = comp[0].reshape(P, COLS)
        pts[:, 1, :] = comp[1].reshape(P, COLS)
        pts[:, 2, :] = comp[2].reshape(P, COLS)
        pts[:, 3, :] = comp[3].reshape(P, COLS)
        in_maps.append(
            {
                "pts": np.ascontiguousarray(pts.reshape(P, 4 * COLS)),
                "dep": np.ascontiguousarray(depth[b, 0].reshape(P, 512)),
            }
        )
    return in_maps


def _combine_outputs(results):
    sq = np.float32(0.0)
    cnt = np.float32(0.0)
    for c in range(B):
        o = results[c]["out"]
        sq += o[:, 0].sum(dtype=np.float64).astype(np.float32)
        cnt += o[:, 1].sum(dtype=np.float64).astype(np.float32)
    loss = sq / max(cnt, np.float32(1.0))
    return np.asarray(loss, dtype=np.float32)


def kernel(points, densities, depth):
    global last_results
    from concourse.bass_utils import run_bass_kernel_spmd

    nc = _get_module()
    in_maps = _prepare_in_maps(points, densities, depth)
    last_results = run_bass_kernel_spmd(nc, in_maps, core_ids=list(range(B)))
    return _combine_outputs(last_results.results)

